# revision 2
# baseline (speedup 1.0000x reference)
"""Trainium2 Bass kernel for the diagonal-Radon problem (quad-tile ap_gather).

One index fetches an 8x8 pixel tile at (4,4)-alignment covering FOUR
consecutive samples (corner footprint span <= 5 rows/cols). Lanes =
(h=2 row-halves of 4 rows, b=8); d=32 bf16 per lane. 31x31=961 entries.
Weight tiles are 8x8 per quad, lane-split the same way (host-replicated
over the 8 batch lanes); DVE does mul + two-stage reduce; the host sums
the two row-half partials.
"""

import numpy as np
import ml_dtypes

BF16 = ml_dtypes.bfloat16

N = 128
B = 8
C = 128
A = 180
C0 = np.float32(63.5)
NT = 31
NE = NT * NT       # 961
M = N // 4         # 32 quads
KA = 12
NCH = A // KA
NCORES = 8
NPASS = 2
JPP = 8
SPP = A * M        # 5760 idxs per channel per pass

LAST_RESULT = None
_prog_cache = {}


def _build_program(reps=1):
    import concourse.bacc as bacc
    import concourse.mybir as mybir
    import concourse.tile as tile

    nc = bacc.Bacc("TRN2", target_bir_lowering=False, debug=False,
                   num_devices=NCORES)
    f32 = mybir.dt.float32
    bf16 = mybir.dt.bfloat16
    i16 = mybir.dt.int16

    xs_in = [nc.dram_tensor(f"xs{h}", [128, NE * 32], bf16,
                            kind="ExternalInput").ap() for h in range(NPASS)]
    idx_in = [nc.dram_tensor(f"idx{h}", [128, SPP // 16], i16,
                             kind="ExternalInput").ap() for h in range(NPASS)]
    wq_in = [nc.dram_tensor(f"wq{h}", [128, SPP * 32], bf16,
                            kind="ExternalInput").ap() for h in range(NPASS)]
    res_out = [nc.dram_tensor(f"res{h}", [128, A], f32,
                              kind="ExternalOutput").ap() for h in range(NPASS)]

    ni = KA * M              # 384 idxs per chunk
    nw = ni * 32             # 12288 weight elems per chunk per lane
    with tile.TileContext(nc) as tc:
        with tc.tile_pool(name="xsp", bufs=1) as xsp, \
             tc.tile_pool(name="idxp", bufs=2) as idxp, \
             tc.tile_pool(name="wqp", bufs=2) as wqp, \
             tc.tile_pool(name="gp", bufs=2) as gp, \
             tc.tile_pool(name="pp", bufs=2) as pp, \
             tc.tile_pool(name="resp", bufs=2) as resp:
          with nc.allow_low_precision(reason="bf16 partials"):
            for _rep in range(reps):
                for h in range(NPASS):
                    xs_t = xsp.tile([128, NE * 32], bf16)
                    nc.sync.dma_start(xs_t[:], xs_in[h])
                    idx_t = idxp.tile([128, SPP // 16], i16)
                    nc.sync.dma_start(idx_t[:], idx_in[h])
                    res_t = resp.tile([128, A], f32)
                    for k in range(NCH):
                        wq_t = wqp.tile([128, nw], bf16)
                        nc.sync.dma_start(
                            wq_t[:], wq_in[h][:, k * nw:(k + 1) * nw])
                        g_t = gp.tile([128, nw], bf16)
                        nc.gpsimd.ap_gather(
                            out_ap=g_t[:].rearrange("p (n d) -> p n d", d=32),
                            in_ap=xs_t[:].rearrange("p (n d) -> p n d", d=32),
                            idxs_ap=idx_t[:, k * (ni // 16):(k + 1) * (ni // 16)],
                            channels=128,
                            num_elems=NE,
                            d=32,
                            num_idxs=ni,
                        )
                        nc.vector.tensor_mul(g_t[:], g_t[:], wq_t[:])
                        p_t = pp.tile([128, ni], bf16)
                        nc.vector.tensor_reduce(
                            p_t[:],
                            g_t[:].rearrange("p (n d) -> p n d", d=32),
                            axis=mybir.AxisListType.X,
                            op=mybir.AluOpType.add,
                            opt_input=False,
                        )
                        nc.vector.tensor_reduce(
                            res_t[:, k * KA:(k + 1) * KA],
                            p_t[:].rearrange("p (a m) -> p a m", m=M),
                            axis=mybir.AxisListType.X,
                            op=mybir.AluOpType.add,
                            opt_input=False,
                        )
                    nc.sync.dma_start(res_out[h], res_t[:])
    nc.compile()
    return nc


def _host_tables(angles):
    ang = np.asarray(angles, dtype=np.float32)
    cosv = np.cos(ang).astype(np.float32)
    sinv = np.sin(ang).astype(np.float32)
    jj = (np.arange(C, dtype=np.float32) - C0)[:, None, None]
    tt = (np.arange(N, dtype=np.float32) - C0)[None, None, :]
    u = (C0 + jj * cosv[None, :, None]) - tt * sinv[None, :, None]
    v = (C0 + jj * sinv[None, :, None]) + tt * cosv[None, :, None]
    u0 = np.floor(u)
    v0 = np.floor(v)
    wu = (u - u0).astype(np.float32)
    wv = (v - v0).astype(np.float32)
    p0 = u0.astype(np.int64)
    q0 = v0.astype(np.int64)

    p0g = p0.reshape(C, A, M, 4)
    q0g = q0.reshape(C, A, M, 4)
    pmin = p0g.min(axis=3)
    qmin = q0g.min(axis=3)
    wr = np.clip(4 * (pmin // 4), 0, 120)
    wc = np.clip(4 * (qmin // 4), 0, 120)
    idx = ((wr // 4) * NT + (wc // 4)).astype(np.int16)

    one = np.float32(1.0)
    w8 = np.zeros((C, A, M, 8, 8), dtype=np.float32)
    wrs = wr.reshape(C, A, M, 1)
    wcs = wc.reshape(C, A, M, 1)
    wug = wu.reshape(C, A, M, 4)
    wvg = wv.reshape(C, A, M, 4)
    flat = w8.reshape(C * A * M, 64)
    base = np.arange(C * A * M).repeat(4)
    for dr in range(2):
        r = p0g + dr
        wrow = (one - wug) if dr == 0 else wug
        rok = (r >= 0) & (r < N)
        for dc in range(2):
            c = q0g + dc
            wcol = (one - wvg) if dc == 0 else wvg
            cok = (c >= 0) & (c < N)
            wgt = (wrow * wcol) * (rok & cok).astype(np.float32)
            rr = r - wrs
            cc = c - wcs
            inside = (rr >= 0) & (rr < 8) & (cc >= 0) & (cc < 8)
            wgt = np.where(inside, wgt, np.float32(0.0))
            rr = np.clip(rr, 0, 7)
            cc = np.clip(cc, 0, 7)
            np.add.at(flat, (base, (rr * 8 + cc).reshape(-1)),
                      wgt.reshape(-1))
    return idx, w8


def _core_inputs(X_bf, idx, w8, core):
    ins = {}
    for h in range(NPASS):
        jsel = core * (NPASS * JPP) + h * JPP + np.arange(JPP)

        xs = np.empty((JPP, 2, B, NE, 32), dtype=BF16)
        for g in range(JPP):
            img = X_bf[:, jsel[g]]                        # [B,128,128]
            s = img.strides
            tiles = np.lib.stride_tricks.as_strided(
                img, shape=(B, NT, NT, 8, 8),
                strides=(s[0], 4 * s[1], 4 * s[2], s[1], s[2]))
            for hh in range(2):
                xs[g, hh] = tiles[:, :, :, 4 * hh:4 * hh + 4, :].reshape(
                    B, NE, 32)
        ins[f"xs{h}"] = xs.reshape(128, NE * 32)

        idxw = np.empty((JPP, 16, SPP // 16), dtype=np.int16)
        for g in range(JPP):
            stream = idx[jsel[g]].reshape(SPP)
            idxw[g] = stream.reshape(SPP // 16, 16).T
        ins[f"idx{h}"] = idxw.reshape(128, SPP // 16)

        sub = w8[jsel]                                    # [JPP,A,M,8,8]
        wq = np.empty((JPP, 2, B, SPP * 32), dtype=BF16)
        for hh in range(2):
            lane = sub[:, :, :, 4 * hh:4 * hh + 4, :].reshape(
                JPP, 1, SPP * 32)
            wq[:, hh] = np.broadcast_to(
                lane, (JPP, B, SPP * 32)).astype(BF16)
        ins[f"wq{h}"] = wq.reshape(128, SPP * 32)
    return ins


def kernel(X, angles):
    global LAST_RESULT
    import os
    os.environ["BASS_NEVER_TRACE"] = "1"
    from concourse.bass_utils import run_bass_kernel_spmd

    X = np.ascontiguousarray(np.asarray(X, dtype=np.float32))
    X_bf = X.astype(BF16)
    if "nc" not in _prog_cache:
        _prog_cache["nc"] = _build_program()
    nc = _prog_cache["nc"]

    akey = np.asarray(angles, dtype=np.float32).tobytes()
    if _prog_cache.get("akey") != akey:
        _prog_cache["tables"] = _host_tables(angles)
        _prog_cache["akey"] = akey
    idx, w8 = _prog_cache["tables"]
    in_maps = [_core_inputs(X_bf, idx, w8, c) for c in range(NCORES)]
    _prog_cache["in_maps"] = in_maps

    result = run_bass_kernel_spmd(
        nc, in_maps, core_ids=list(range(NCORES)), trace=False)
    LAST_RESULT = result

    out = np.zeros((B, C, 1, A), dtype=np.float32)
    for c in range(NCORES):
        for h in range(NPASS):
            res = result.results[c][f"res{h}"].reshape(JPP, 2, B, A)
            part = (res[:, 0].astype(np.float32)
                    + res[:, 1].astype(np.float32))
            jsel = c * (NPASS * JPP) + h * JPP + np.arange(JPP)
            out[:, jsel, 0, :] = part.transpose(1, 0, 2)
    return out


def emulate(X, angles):
    """Numpy emulation (core 0 only is enough to validate tables/layout,
    but run all cores for the full-tensor check)."""
    X_bf = np.asarray(X, np.float32).astype(BF16)
    idx, w8 = _host_tables(angles)
    out = np.zeros((B, C, 1, A), dtype=np.float32)
    for c in range(NCORES):
        ins = _core_inputs(X_bf, idx, w8, c)
        for h in range(NPASS):
            xs = ins[f"xs{h}"].reshape(128, NE, 32)
            idxw = ins[f"idx{h}"].reshape(JPP, 16, SPP // 16)
            wqc = ins[f"wq{h}"].reshape(128, SPP, 32)
            res = np.zeros((128, A), np.float32)
            for g in range(JPP):
                stream = idxw[g].T.reshape(SPP).astype(np.int64)
                for lane in range(16):
                    p = g * 16 + lane
                    gat = xs[p][stream]
                    prod = (gat.astype(np.float32)
                            * wqc[p].astype(np.float32)).astype(BF16)
                    psum = prod.astype(np.float32).sum(axis=1).astype(BF16)
                    res[p] = psum.astype(np.float32).reshape(A, M).sum(axis=1)
            resr = res.reshape(JPP, 2, B, A)
            part = resr[:, 0] + resr[:, 1]
            jsel = c * (NPASS * JPP) + h * JPP + np.arange(JPP)
            out[:, jsel, 0, :] = part.transpose(1, 0, 2)
    return out


# ---------------------------------------------------------------------------
# Timing support (no NTFF profiling hook in this environment): slope method.
# ---------------------------------------------------------------------------

def _make_sharded_callable(nc):
    import jax
    from jax.sharding import Mesh, PartitionSpec, NamedSharding
    from jax.experimental.shard_map import shard_map
    import concourse.mybir as mybir
    import concourse.bass2jax as bass2jax

    bass2jax.install_neuronx_cc_hook()

    partition_name = (nc.partition_id_tensor.name
                      if nc.partition_id_tensor else None)
    in_names, out_names, out_avals, zero_outs = [], [], [], []
    for alloc in nc.m.functions[0].allocations:
        if not isinstance(alloc, mybir.MemoryLocationSet):
            continue
        name = alloc.memorylocations[0].name
        if alloc.kind == "ExternalInput":
            if name != partition_name:
                in_names.append(name)
        elif alloc.kind == "ExternalOutput":
            out_names.append(name)
            shape = tuple(alloc.tensor_shape)
            dtype = mybir.dt.np(alloc.dtype)
            out_avals.append(jax.core.ShapedArray(shape, dtype))
            zero_outs.append(np.zeros(shape, dtype))
    n_params = len(in_names)
    all_in_names = list(in_names) + list(out_names)
    if partition_name is not None:
        all_in_names.append(partition_name)

    def _body(*args):
        operands = list(args)
        if partition_name is not None:
            operands.append(bass2jax.partition_id_tensor())
        outs = bass2jax._bass_exec_p.bind(
            *operands,
            out_avals=tuple(out_avals),
            in_names=tuple(all_in_names),
            out_names=tuple(out_names),
            lowering_input_output_aliases=(),
            sim_require_finite=True,
            sim_require_nnan=True,
            nc=nc,
        )
        return tuple(outs)

    devices = jax.devices()[:NCORES]
    mesh = Mesh(np.asarray(devices), ("core",))
    spec = PartitionSpec("core")
    in_specs = (spec,) * (n_params + len(out_names))
    out_specs = (spec,) * len(out_names)
    donate = tuple(range(n_params, n_params + len(out_names)))
    fn = jax.jit(
        shard_map(_body, mesh=mesh, in_specs=in_specs, out_specs=out_specs,
                  check_rep=False),
        donate_argnums=donate, keep_unused=True)
    sharding = NamedSharding(mesh, spec)
    return fn, in_names, zero_outs, sharding


def _timed_exec(nc, in_maps, iters):
    import time
    import jax

    fn, in_names, zero_outs, sharding = _make_sharded_callable(nc)
    concat_in = [
        jax.device_put(
            np.concatenate([np.asarray(in_maps[c][n]) for c in range(NCORES)],
                           axis=0), sharding)
        for n in in_names
    ]

    def one_call():
        zeros = [
            jax.device_put(
                np.zeros((NCORES * z.shape[0], *z.shape[1:]), z.dtype),
                sharding)
            for z in zero_outs
        ]
        for z in zeros:
            z.block_until_ready()
        t0 = time.monotonic()
        outs = fn(*concat_in, *zeros)
        for o in outs:
            o.block_until_ready()
        return time.monotonic() - t0

    one_call()  # compile + warm
    times = [one_call() for _ in range(iters)]
    return float(np.median(times)), times


def measure_hw_time_ns(iters=25, reps=17):
    """Estimated on-device exec time via the slope method."""
    nc1 = _prog_cache.get("nc")
    in_maps = _prog_cache.get("in_maps")
    if nc1 is None or in_maps is None:
        raise RuntimeError("run kernel() first")
    key = f"ncR{reps}"
    if key not in _prog_cache:
        _prog_cache[key] = _build_program(reps=reps)
    ncR = _prog_cache[key]
    _, t1_all = _timed_exec(nc1, in_maps, iters)
    _, tR_all = _timed_exec(ncR, in_maps, iters)
    t1 = min(t1_all)
    tR = min(tR_all)
    est = (tR - t1) / (reps - 1)
    return (est * 1e9, t1 * 1e9, tR * 1e9,
            [t * 1e9 for t in t1_all], [t * 1e9 for t in tR_all])



# revision 3
# speedup vs baseline: 1.2536x; 1.2536x over previous
"""Trainium2 Bass kernel for the diagonal-Radon problem (quad-tile ap_gather).

One index fetches an 8x8 pixel tile at (4,4)-alignment covering FOUR
consecutive samples (corner footprint span <= 5 rows/cols). Lanes =
(h=2 row-halves of 4 rows, b=8); d=32 bf16 per lane. 31x31=961 entries.
Weight tiles are 8x8 per quad, lane-split the same way (host-replicated
over the 8 batch lanes); DVE does mul + two-stage reduce; the host sums
the two row-half partials.
"""

import numpy as np
import ml_dtypes

BF16 = ml_dtypes.bfloat16

N = 128
B = 8
C = 128
A = 180
C0 = np.float32(63.5)
NT = 31
NE = NT * NT       # 961
M = N // 4         # 32 quads
KA = 12
NCH = A // KA
NCORES = 8
NPASS = 2
JPP = 8
SPP = A * M        # 5760 idxs per channel per pass

LAST_RESULT = None
_prog_cache = {}


def _build_program(reps=1):
    import concourse.bacc as bacc
    import concourse.mybir as mybir
    import concourse.tile as tile

    nc = bacc.Bacc("TRN2", target_bir_lowering=False, debug=False,
                   num_devices=NCORES)
    f32 = mybir.dt.float32
    bf16 = mybir.dt.bfloat16
    i16 = mybir.dt.int16

    xs_in = [nc.dram_tensor(f"xs{h}", [128, NE * 32], bf16,
                            kind="ExternalInput").ap() for h in range(NPASS)]
    idx_in = [nc.dram_tensor(f"idx{h}", [128, SPP // 16], i16,
                             kind="ExternalInput").ap() for h in range(NPASS)]
    wq_in = [nc.dram_tensor(f"wq{h}", [128, SPP * 32], bf16,
                            kind="ExternalInput").ap() for h in range(NPASS)]
    res_out = [nc.dram_tensor(f"res{h}", [128, A], f32,
                              kind="ExternalOutput").ap() for h in range(NPASS)]

    ni = KA * M              # 384 idxs per chunk
    nw = ni * 32             # 12288 weight elems per chunk per lane
    with tile.TileContext(nc) as tc:
        with tc.tile_pool(name="xsp", bufs=1) as xsp, \
             tc.tile_pool(name="idxp", bufs=2) as idxp, \
             tc.tile_pool(name="wqp", bufs=2) as wqp, \
             tc.tile_pool(name="gp", bufs=3) as gp, \
             tc.tile_pool(name="pp", bufs=2) as pp, \
             tc.tile_pool(name="resp", bufs=2) as resp:
          with nc.allow_low_precision(reason="bf16 partials"):
            for _rep in range(reps):
                for h in range(NPASS):
                    xs_t = xsp.tile([128, NE * 32], bf16)
                    nc.sync.dma_start(xs_t[:], xs_in[h])
                    idx_t = idxp.tile([128, SPP // 16], i16)
                    nc.sync.dma_start(idx_t[:], idx_in[h])
                    res_t = resp.tile([128, A], f32)
                    for k in range(NCH):
                        wq_t = wqp.tile([128, nw], bf16)
                        nc.sync.dma_start(
                            wq_t[:], wq_in[h][:, k * nw:(k + 1) * nw])
                        g_t = gp.tile([128, nw], bf16)
                        nc.gpsimd.ap_gather(
                            out_ap=g_t[:].rearrange("p (n d) -> p n d", d=32),
                            in_ap=xs_t[:].rearrange("p (n d) -> p n d", d=32),
                            idxs_ap=idx_t[:, k * (ni // 16):(k + 1) * (ni // 16)],
                            channels=128,
                            num_elems=NE,
                            d=32,
                            num_idxs=ni,
                        )
                        nc.vector.tensor_mul(g_t[:], g_t[:], wq_t[:])
                        nc.vector.tensor_reduce(
                            res_t[:, k * KA:(k + 1) * KA],
                            g_t[:].rearrange("p (a x) -> p a x", x=M * 32),
                            axis=mybir.AxisListType.X,
                            op=mybir.AluOpType.add,
                            opt_input=False,
                        )
                    nc.sync.dma_start(res_out[h], res_t[:])
    nc.compile()
    return nc


def _host_tables(angles):
    ang = np.asarray(angles, dtype=np.float32)
    cosv = np.cos(ang).astype(np.float32)
    sinv = np.sin(ang).astype(np.float32)
    jj = (np.arange(C, dtype=np.float32) - C0)[:, None, None]
    tt = (np.arange(N, dtype=np.float32) - C0)[None, None, :]
    u = (C0 + jj * cosv[None, :, None]) - tt * sinv[None, :, None]
    v = (C0 + jj * sinv[None, :, None]) + tt * cosv[None, :, None]
    u0 = np.floor(u)
    v0 = np.floor(v)
    wu = (u - u0).astype(np.float32)
    wv = (v - v0).astype(np.float32)
    p0 = u0.astype(np.int64)
    q0 = v0.astype(np.int64)

    p0g = p0.reshape(C, A, M, 4)
    q0g = q0.reshape(C, A, M, 4)
    pmin = p0g.min(axis=3)
    qmin = q0g.min(axis=3)
    wr = np.clip(4 * (pmin // 4), 0, 120)
    wc = np.clip(4 * (qmin // 4), 0, 120)
    idx = ((wr // 4) * NT + (wc // 4)).astype(np.int16)

    one = np.float32(1.0)
    w8 = np.zeros((C, A, M, 8, 8), dtype=np.float32)
    wrs = wr.reshape(C, A, M, 1)
    wcs = wc.reshape(C, A, M, 1)
    wug = wu.reshape(C, A, M, 4)
    wvg = wv.reshape(C, A, M, 4)
    flat = w8.reshape(C * A * M, 64)
    base = np.arange(C * A * M).repeat(4)
    for dr in range(2):
        r = p0g + dr
        wrow = (one - wug) if dr == 0 else wug
        rok = (r >= 0) & (r < N)
        for dc in range(2):
            c = q0g + dc
            wcol = (one - wvg) if dc == 0 else wvg
            cok = (c >= 0) & (c < N)
            wgt = (wrow * wcol) * (rok & cok).astype(np.float32)
            rr = r - wrs
            cc = c - wcs
            inside = (rr >= 0) & (rr < 8) & (cc >= 0) & (cc < 8)
            wgt = np.where(inside, wgt, np.float32(0.0))
            rr = np.clip(rr, 0, 7)
            cc = np.clip(cc, 0, 7)
            np.add.at(flat, (base, (rr * 8 + cc).reshape(-1)),
                      wgt.reshape(-1))
    return idx, w8


def _core_inputs(X_bf, idx, w8, core):
    ins = {}
    for h in range(NPASS):
        jsel = core * (NPASS * JPP) + h * JPP + np.arange(JPP)

        xs = np.empty((JPP, 2, B, NE, 32), dtype=BF16)
        for g in range(JPP):
            img = X_bf[:, jsel[g]]                        # [B,128,128]
            s = img.strides
            tiles = np.lib.stride_tricks.as_strided(
                img, shape=(B, NT, NT, 8, 8),
                strides=(s[0], 4 * s[1], 4 * s[2], s[1], s[2]))
            for hh in range(2):
                xs[g, hh] = tiles[:, :, :, 4 * hh:4 * hh + 4, :].reshape(
                    B, NE, 32)
        ins[f"xs{h}"] = xs.reshape(128, NE * 32)

        idxw = np.empty((JPP, 16, SPP // 16), dtype=np.int16)
        for g in range(JPP):
            stream = idx[jsel[g]].reshape(SPP)
            idxw[g] = stream.reshape(SPP // 16, 16).T
        ins[f"idx{h}"] = idxw.reshape(128, SPP // 16)

        sub = w8[jsel]                                    # [JPP,A,M,8,8]
        wq = np.empty((JPP, 2, B, SPP * 32), dtype=BF16)
        for hh in range(2):
            lane = sub[:, :, :, 4 * hh:4 * hh + 4, :].reshape(
                JPP, 1, SPP * 32)
            wq[:, hh] = np.broadcast_to(
                lane, (JPP, B, SPP * 32)).astype(BF16)
        ins[f"wq{h}"] = wq.reshape(128, SPP * 32)
    return ins


def kernel(X, angles):
    global LAST_RESULT
    import os
    os.environ["BASS_NEVER_TRACE"] = "1"
    from concourse.bass_utils import run_bass_kernel_spmd

    X = np.ascontiguousarray(np.asarray(X, dtype=np.float32))
    X_bf = X.astype(BF16)
    if "nc" not in _prog_cache:
        _prog_cache["nc"] = _build_program()
    nc = _prog_cache["nc"]

    akey = np.asarray(angles, dtype=np.float32).tobytes()
    if _prog_cache.get("akey") != akey:
        _prog_cache["tables"] = _host_tables(angles)
        _prog_cache["akey"] = akey
    idx, w8 = _prog_cache["tables"]
    in_maps = [_core_inputs(X_bf, idx, w8, c) for c in range(NCORES)]
    _prog_cache["in_maps"] = in_maps

    result = run_bass_kernel_spmd(
        nc, in_maps, core_ids=list(range(NCORES)), trace=False)
    LAST_RESULT = result

    out = np.zeros((B, C, 1, A), dtype=np.float32)
    for c in range(NCORES):
        for h in range(NPASS):
            res = result.results[c][f"res{h}"].reshape(JPP, 2, B, A)
            part = (res[:, 0].astype(np.float32)
                    + res[:, 1].astype(np.float32))
            jsel = c * (NPASS * JPP) + h * JPP + np.arange(JPP)
            out[:, jsel, 0, :] = part.transpose(1, 0, 2)
    return out


def emulate(X, angles):
    """Numpy emulation (core 0 only is enough to validate tables/layout,
    but run all cores for the full-tensor check)."""
    X_bf = np.asarray(X, np.float32).astype(BF16)
    idx, w8 = _host_tables(angles)
    out = np.zeros((B, C, 1, A), dtype=np.float32)
    for c in range(NCORES):
        ins = _core_inputs(X_bf, idx, w8, c)
        for h in range(NPASS):
            xs = ins[f"xs{h}"].reshape(128, NE, 32)
            idxw = ins[f"idx{h}"].reshape(JPP, 16, SPP // 16)
            wqc = ins[f"wq{h}"].reshape(128, SPP, 32)
            res = np.zeros((128, A), np.float32)
            for g in range(JPP):
                stream = idxw[g].T.reshape(SPP).astype(np.int64)
                for lane in range(16):
                    p = g * 16 + lane
                    gat = xs[p][stream]
                    prod = (gat.astype(np.float32)
                            * wqc[p].astype(np.float32)).astype(BF16)
                    psum = prod.astype(np.float32).sum(axis=1).astype(BF16)
                    res[p] = psum.astype(np.float32).reshape(A, M).sum(axis=1)
            resr = res.reshape(JPP, 2, B, A)
            part = resr[:, 0] + resr[:, 1]
            jsel = c * (NPASS * JPP) + h * JPP + np.arange(JPP)
            out[:, jsel, 0, :] = part.transpose(1, 0, 2)
    return out


# ---------------------------------------------------------------------------
# Timing support (no NTFF profiling hook in this environment): slope method.
# ---------------------------------------------------------------------------

def _make_sharded_callable(nc):
    import jax
    from jax.sharding import Mesh, PartitionSpec, NamedSharding
    from jax.experimental.shard_map import shard_map
    import concourse.mybir as mybir
    import concourse.bass2jax as bass2jax

    bass2jax.install_neuronx_cc_hook()

    partition_name = (nc.partition_id_tensor.name
                      if nc.partition_id_tensor else None)
    in_names, out_names, out_avals, zero_outs = [], [], [], []
    for alloc in nc.m.functions[0].allocations:
        if not isinstance(alloc, mybir.MemoryLocationSet):
            continue
        name = alloc.memorylocations[0].name
        if alloc.kind == "ExternalInput":
            if name != partition_name:
                in_names.append(name)
        elif alloc.kind == "ExternalOutput":
            out_names.append(name)
            shape = tuple(alloc.tensor_shape)
            dtype = mybir.dt.np(alloc.dtype)
            out_avals.append(jax.core.ShapedArray(shape, dtype))
            zero_outs.append(np.zeros(shape, dtype))
    n_params = len(in_names)
    all_in_names = list(in_names) + list(out_names)
    if partition_name is not None:
        all_in_names.append(partition_name)

    def _body(*args):
        operands = list(args)
        if partition_name is not None:
            operands.append(bass2jax.partition_id_tensor())
        outs = bass2jax._bass_exec_p.bind(
            *operands,
            out_avals=tuple(out_avals),
            in_names=tuple(all_in_names),
            out_names=tuple(out_names),
            lowering_input_output_aliases=(),
            sim_require_finite=True,
            sim_require_nnan=True,
            nc=nc,
        )
        return tuple(outs)

    devices = jax.devices()[:NCORES]
    mesh = Mesh(np.asarray(devices), ("core",))
    spec = PartitionSpec("core")
    in_specs = (spec,) * (n_params + len(out_names))
    out_specs = (spec,) * len(out_names)
    donate = tuple(range(n_params, n_params + len(out_names)))
    fn = jax.jit(
        shard_map(_body, mesh=mesh, in_specs=in_specs, out_specs=out_specs,
                  check_rep=False),
        donate_argnums=donate, keep_unused=True)
    sharding = NamedSharding(mesh, spec)
    return fn, in_names, zero_outs, sharding


def _timed_exec(nc, in_maps, iters):
    import time
    import jax

    fn, in_names, zero_outs, sharding = _make_sharded_callable(nc)
    concat_in = [
        jax.device_put(
            np.concatenate([np.asarray(in_maps[c][n]) for c in range(NCORES)],
                           axis=0), sharding)
        for n in in_names
    ]

    def one_call():
        zeros = [
            jax.device_put(
                np.zeros((NCORES * z.shape[0], *z.shape[1:]), z.dtype),
                sharding)
            for z in zero_outs
        ]
        for z in zeros:
            z.block_until_ready()
        t0 = time.monotonic()
        outs = fn(*concat_in, *zeros)
        for o in outs:
            o.block_until_ready()
        return time.monotonic() - t0

    one_call()  # compile + warm
    times = [one_call() for _ in range(iters)]
    return float(np.median(times)), times


def measure_hw_time_ns(iters=25, reps=17):
    """Estimated on-device exec time via the slope method."""
    nc1 = _prog_cache.get("nc")
    in_maps = _prog_cache.get("in_maps")
    if nc1 is None or in_maps is None:
        raise RuntimeError("run kernel() first")
    key = f"ncR{reps}"
    if key not in _prog_cache:
        _prog_cache[key] = _build_program(reps=reps)
    ncR = _prog_cache[key]
    _, t1_all = _timed_exec(nc1, in_maps, iters)
    _, tR_all = _timed_exec(ncR, in_maps, iters)
    t1 = min(t1_all)
    tR = min(tR_all)
    est = (tR - t1) / (reps - 1)
    return (est * 1e9, t1 * 1e9, tR * 1e9,
            [t * 1e9 for t in t1_all], [t * 1e9 for t in tR_all])

